# revision 20
# baseline (speedup 1.0000x reference)
"""Trainium2 Bass kernel for nn_ComplexLinearAndLeakyReLU (v2, fp16 pipeline).

Math (per batch b, point c, channel e), reformulated:
  w = (uz, 0, nz) z-column of the orthonormal frame of J, with
    nz = jz/|J|,  uz = -g/sqrt(t1 + g^2),  g = t1/(jz+eps),  t1 = jx^2+jy^2.
  s  = w . X ;  Y = A@X + (Cw-A)@[w*s] + Bw@(X x w)   (contraction over e)
  d = W@x (x = Y);  out = x - 0.8*min(dot,0)/(dns+eps) * d.

Implementation notes:
  - fp16 storage end-to-end (DVE 2x mode, matmul FWL); fp32 islands for
    range-critical basis tensors (jzp, rD, g1, gsq-in-w2) and inv0.
  - dot/dns 3-way reductions run on the TENSOR engine as identity-matmul
    PSUM accumulations (frees DVE/GPS).
  - Engine balance measured from NTFF: DVE = TT stream, ACT = activations
    + dsb cast, GPS = small adds.
  - One batch per NeuronCore (8 cores), weights replicated.
"""

import numpy as np
from contextlib import ExitStack

import concourse.bass as bass
import concourse.tile as tile
from concourse import bacc, mybir
from concourse.bass_utils import run_bass_kernel_spmd

F32 = mybir.dt.float32
F16 = mybir.dt.float16
ALU = mybir.AluOpType
ACTF = mybir.ActivationFunctionType

B, C, E, F = 8, 2048, 256, 256
EPS = 1e-6

# --- tunables -------------------------------------------------------------
CW = 512            # columns per outer chunk (elementwise granularity)
NCH = C // CW
MMN = 512           # matmul moving free size (== CW)
NBUF = dict(inp=2, bas=2, prod=2, xd=2, tail=1, out=2)


def _mk(ap, dims):
    """Build an AP over the same tensor with explicit [stride, size] free dims."""
    return bass.AP(tensor=ap.tensor, offset=ap.offset, ap=[ap.ap[0]] + dims)


def build_nc():
    nc = bacc.Bacc("TRN2", target_bir_lowering=False, debug=False, num_devices=8)

    for val in (EPS, 1.25 * EPS):
        t = nc.alloc_sbuf_tensor(f"const-f32-{val}", [128, 1], F32)
        nc.gpsimd.memset(t.ap(), val)
        nc.const_aps.aps[(F32, val)] = t.ap()
    nc.all_engine_barrier()

    xp = nc.dram_tensor("xp", [3, E, C], F16, kind="ExternalInput")
    jp = nc.dram_tensor("jp", [3, E, C], F16, kind="ExternalInput")
    wy = nc.dram_tensor("wy", [4, E, F], F16, kind="ExternalInput")  # A^T,(Cw-A)^T,Bw^T,(-Bw)^T
    wt = nc.dram_tensor("wt", [F, F], F16, kind="ExternalInput")     # W^T
    ident = nc.dram_tensor("ident", [128, 128], F16, kind="ExternalInput")
    out = nc.dram_tensor("out", [F, 3, C], F16, kind="ExternalOutput")

    with tile.TileContext(nc) as tc, ExitStack() as ctx:
        wpool = ctx.enter_context(tc.tile_pool(name="w", bufs=1))
        inpool = ctx.enter_context(tc.tile_pool(name="inp", bufs=NBUF["inp"]))
        baspool = ctx.enter_context(tc.tile_pool(name="bas", bufs=NBUF["bas"]))
        prodpool = ctx.enter_context(tc.tile_pool(name="prod", bufs=NBUF["prod"]))
        xdpool = ctx.enter_context(tc.tile_pool(name="xd", bufs=NBUF["xd"]))
        tailpool = ctx.enter_context(tc.tile_pool(name="tail", bufs=NBUF["tail"]))
        outpool = ctx.enter_context(tc.tile_pool(name="outp", bufs=NBUF["out"]))
        ypool = ctx.enter_context(tc.tile_pool(name="ypsum", bufs=1, space="PSUM"))
        dpool = ctx.enter_context(tc.tile_pool(name="dpsum", bufs=1, space="PSUM"))
        rpool = dpool  # dall / dnsP / dotP rotate through the same 2-bank slot

        # --- weights: once, resident ---
        wy_sb = []
        for t in range(4):
            w_t = wpool.tile([128, 2, F], F16, tag=f"wy{t}", name=f"wy{t}")
            nc.sync.dma_start(w_t[:], wy[t].rearrange("(k p) f -> p k f", p=128))
            wy_sb.append(w_t)
        wt_sb = wpool.tile([128, 2, F], F16, tag="wt", name="wt")
        nc.sync.dma_start(wt_sb[:], wt.rearrange("(k p) f -> p k f", p=128))
        id_sb = wpool.tile([128, 128], F16, tag="ident", name="ident")
        nc.sync.dma_start(id_sb[:], ident[:, :])

        for ci in range(NCH):
            c0 = ci * CW

            J3 = inpool.tile([128, 3, 2, CW], F16, tag="J3", name="J3")
            X3 = inpool.tile([128, 3, 2, CW], F16, tag="X3", name="X3")
            nc.sync.dma_start(
                J3[:], jp[:, :, c0:c0 + CW].rearrange("i (k p) c -> p i k c", p=128)
            )
            nc.scalar.dma_start(
                X3[:], xp[:, :, c0:c0 + CW].rearrange("i (k p) c -> p i k c", p=128)
            )
            jz = J3[:, 2]

            def BT(tag, dt=F16, shape=None):
                return baspool.tile(shape or [128, 2, CW], dt, tag=tag, name=tag)

            # --- basis ---
            # ramp: first chunks' basis chain is latency-critical while DVE is
            # still idle -> run the slow-GPS hops on DVE there instead.
            ramp = nc.vector if ci < 1 else nc.gpsimd
            q123 = BT("q123", shape=[128, 3, 2, CW])
            nc.scalar.activation(q123[:], J3[:], ACTF.Square)
            t1 = BT("t1"); ramp.tensor_add(t1[:], q123[:, 0], q123[:, 1])
            n2 = BT("n2"); ramp.tensor_add(n2[:], t1[:], q123[:, 2])
            i1 = BT("i1"); nc.scalar.activation(i1[:], n2[:], ACTF.Abs_reciprocal_sqrt)
            jzp = BT("jzp", F32)
            if ci < 1:
                nc.vector.tensor_scalar_add(jzp[:], jz, EPS)
            else:
                nc.scalar.activation(jzp[:], jz, ACTF.Identity, bias=EPS)
            rD = BT("rD", F32); nc.vector.reciprocal_approx_fast(rD[:], jzp[:])
            g1 = BT("g1", F32); nc.vector.tensor_mul(g1[:], t1[:], rD[:])
            gsq = BT("gsq", F32); nc.scalar.square(gsq[:], g1[:])
            w2 = BT("jzp", F32); ramp.tensor_add(w2[:], t1[:], gsq[:])  # reuse jzp slot
            i3 = BT("i3"); nc.scalar.activation(i3[:], w2[:], ACTF.Abs_reciprocal_sqrt)
            # uznz: [which(uz,nz), k, c]
            uznz = baspool.tile([128, 2, 2, CW], F16, tag="uznz", name="uznz")
            nc.vector.scalar_tensor_tensor(uznz[:, 0], g1[:], -1.0, i3[:], ALU.mult, ALU.mult)
            nc.vector.tensor_mul(uznz[:, 1], jz, i1[:])

            # --- products ---
            # UN6[w, i, kc] = uznz[w] * X3[i]
            UN6 = prodpool.tile([128, 2, 3, 2, CW], F16, tag="UN6", name="UN6")
            KC = 2 * CW
            un_in0 = _mk(uznz[:], [[KC, 2], [0, 3], [1, KC]])
            un_in1 = _mk(X3[:], [[0, 2], [KC, 3], [1, KC]])
            un_out = _mk(UN6[:], [[3 * KC, 2], [KC, 3], [1, KC]])
            nc.vector.tensor_tensor(un_out, un_in0, un_in1, ALU.mult)
            # s = uz*xx + nz*xz
            s = prodpool.tile([128, 2, CW], F16, tag="s", name="s")
            ramp.tensor_add(s[:], UN6[:, 0, 0], UN6[:, 1, 2])
            # Q1 = uz*xz - nz*xx (one DVE op beats an extra GEMM term)
            Q1 = prodpool.tile([128, 2, CW], F16, tag="Q1", name="Q1")
            nc.vector.tensor_sub(Q1[:], UN6[:, 0, 2], UN6[:, 1, 0])
            # P02[w] = uznz[w] * s
            P02 = prodpool.tile([128, 2, 2, CW], F16, tag="P02", name="P02")
            p_in0 = _mk(uznz[:], [[KC, 2], [1, KC]])
            p_in1 = _mk(s[:], [[0, 2], [1, KC]])
            p_out = _mk(P02[:], [[KC, 2], [1, KC]])
            nc.vector.tensor_tensor(p_out, p_in0, p_in1, ALU.mult)

            xx, xy, xz = X3[:, 0], X3[:, 1], X3[:, 2]
            P0, P2 = P02[:, 0], P02[:, 1]
            Q0 = UN6[:, 1, 1]; Q2 = UN6[:, 0, 1]

            # --- Y GEMMs, emitted shallow-deps-first so the in-order PE queue
            # streams A-terms while the basis/product chain is still running ---
            ordered = [
                (0, 0, xx), (1, 0, xy), (2, 0, xz),     # A-terms: X3 only
                (0, 2, Q0), (1, 2, Q1), (2, 3, Q2),     # Bw-terms: need uz/nz
                (0, 1, P0), (2, 1, P2),                 # (Cw-A)-terms: deepest
            ]
            n_mm = {0: 6, 1: 4, 2: 6}   # MMs per (i, fj) psum slice
            cnt = {(i, fj): 0 for i in range(3) for fj in range(2)}
            yall = ypool.tile([128, 3, 2, MMN], F32, tag="yall", name="yall")
            for (i, tw, plane) in ordered:
                for fj in range(2):
                    for ke in range(2):
                        k = cnt[(i, fj)]
                        nc.tensor.matmul(
                            yall[:, i, fj, :],
                            lhsT=wy_sb[tw][:, ke, fj * 128:(fj + 1) * 128],
                            rhs=plane[:, ke, :],
                            start=(k == 0), stop=(k == n_mm[i] - 1),
                        )
                        cnt[(i, fj)] = k + 1
            xall = xdpool.tile([128, 3, 2, CW], F16, tag="xall", name="xall")
            dsb = xdpool.tile([128, 3, 2, CW], F16, tag="dsb", name="dsb")
            for i in range(3):
                nc.vector.tensor_copy(xall[:, i], yall[:, i])
                dall = dpool.tile([128, 2, MMN], F32, tag="dr", name="dall")
                for fj in range(2):
                    for kg in range(2):
                        nc.tensor.matmul(
                            dall[:, fj, :],
                            lhsT=wt_sb[:, kg, fj * 128:(fj + 1) * 128],
                            rhs=xall[:, i, kg, :],
                            start=(kg == 0), stop=(kg == 1),
                        )
                nc.scalar.copy(dsb[:, i], dall[:])

            # --- VN-LeakyReLU tail ---
            def TT(tag, dt=F16, shape=None):
                return tailpool.tile(shape or [128, 2, CW], dt, tag=tag, name=tag)

            ea = TT("ea", shape=[128, 3, 2, CW])
            nc.vector.tensor_mul(ea[:], dsb[:], dsb[:])
            dva = TT("dva", shape=[128, 3, 2, CW])
            nc.vector.tensor_mul(dva[:], xall[:], dsb[:])
            # dot/dns: 3-way reductions as identity-matmul PSUM accumulation.
            # dns first: dotP reuses the shared psum slot after sqW read dnsP.
            dnsP = rpool.tile([128, 2 * CW], F32, tag="dr", name="dnsP")
            for fj in range(2):
                for i in range(3):
                    nc.tensor.matmul(
                        dnsP[:, fj * CW:(fj + 1) * CW], lhsT=id_sb[:],
                        rhs=ea[:, i, fj, :], start=(i == 0), stop=(i == 2),
                    )
            # inv0 = 0.8/(dns+eps) = 1/sqrt((1.25*dns+1.25*eps)^2)
            sqW = TT("sqW", F32)
            nc.scalar.activation(sqW[:], _mk(dnsP[:], [[CW, 2], [1, CW]]),
                                 ACTF.Square, bias=1.25 * EPS, scale=1.25)
            inv0 = TT("inv0", F32)
            nc.scalar.activation(inv0[:], sqW[:], ACTF.Abs_reciprocal_sqrt)
            dotP = rpool.tile([128, 2 * CW], F32, tag="dr", name="dotP")
            for fj in range(2):
                for i in range(3):
                    nc.tensor.matmul(
                        dotP[:, fj * CW:(fj + 1) * CW], lhsT=id_sb[:],
                        rhs=dva[:, i, fj, :], start=(i == 0), stop=(i == 2),
                    )
            # rr = min(dot,0)*inv0  (<= 0);  out = x - (rr*d)
            rr = TT("rr")
            nc.vector.scalar_tensor_tensor(
                rr[:], _mk(dotP[:], [[CW, 2], [1, CW]]), 0.0, inv0[:], ALU.min, ALU.mult
            )
            ga = TT("ga", shape=[128, 3, 2, CW])
            g_in0 = _mk(rr[:], [[0, 3], [1, KC]])
            g_in1 = _mk(dsb[:], [[KC, 3], [1, KC]])
            g_out = _mk(ga[:], [[KC, 3], [1, KC]])
            nc.vector.tensor_tensor(g_out, g_in0, g_in1, ALU.mult)
            oall = outpool.tile([128, 3, 2, CW], F16, tag="oall", name="oall")
            nc.vector.tensor_sub(oall[:], xall[:], ga[:])
            nc.sync.dma_start(
                out[:, :, c0:c0 + CW].rearrange("(k p) i c -> p i k c", p=128), oall[:]
            )

    nc.compile()
    return nc


_NC_CACHE = {}


def _get_nc():
    if "nc" not in _NC_CACHE:
        _NC_CACHE["nc"] = build_nc()
    return _NC_CACHE["nc"]


def _prep_inputs(X, J, A, Bw, Cw, W):
    A = np.asarray(A, dtype=np.float32)
    Bw = np.asarray(Bw, dtype=np.float32)
    Cw = np.asarray(Cw, dtype=np.float32)
    W = np.asarray(W, dtype=np.float32)
    wy = np.ascontiguousarray(
        np.stack([A.T, (Cw - A).T, Bw.T, (-Bw).T])
    ).astype(np.float16)                       # [4, E, F]
    wt = np.ascontiguousarray(W.T).astype(np.float16)
    ident = np.eye(128, dtype=np.float16)
    in_maps = []
    for b in range(B):
        in_maps.append({
            "xp": np.ascontiguousarray(np.asarray(X[b]).transpose(2, 1, 0)).astype(np.float16),
            "jp": np.ascontiguousarray(np.asarray(J[b]).transpose(2, 1, 0)).astype(np.float16),
            "wy": wy,
            "wt": wt,
            "ident": ident,
        })
    return in_maps


def kernel(X, J, A, Bw, Cw, W):
    in_maps = _prep_inputs(X, J, A, Bw, Cw, W)
    nc = _get_nc()
    try:
        res = run_bass_kernel_spmd(nc, in_maps, core_ids=list(range(B)))
    except Exception:
        import time as _time
        _time.sleep(15)  # transient NRT device errors recover on retry
        res = run_bass_kernel_spmd(nc, in_maps, core_ids=list(range(B)))
    return np.stack([np.asarray(res.results[b]["out"]).astype(np.float32) for b in range(B)])


# revision 21
# speedup vs baseline: 1.0525x; 1.0525x over previous
"""Trainium2 Bass kernel for nn_ComplexLinearAndLeakyReLU (v2, fp16 pipeline).

Math (per batch b, point c, channel e), reformulated:
  w = (uz, 0, nz) z-column of the orthonormal frame of J, with
    nz = jz/|J|,  uz = -g/sqrt(t1 + g^2),  g = t1/(jz+eps),  t1 = jx^2+jy^2.
  s  = w . X ;  Y = A@X + (Cw-A)@[w*s] + Bw@(X x w)   (contraction over e)
  d = W@x (x = Y);  out = x - 0.8*min(dot,0)/(dns+eps) * d.

Implementation notes:
  - fp16 storage end-to-end (DVE 2x mode, matmul FWL); fp32 islands for
    range-critical basis tensors (jzp, rD, g1, gsq-in-w2) and inv0.
  - dot/dns 3-way reductions run on the TENSOR engine as identity-matmul
    PSUM accumulations (frees DVE/GPS).
  - Engine balance measured from NTFF: DVE = TT stream, ACT = activations
    + dsb cast, GPS = small adds.
  - One batch per NeuronCore (8 cores), weights replicated.
"""

import numpy as np
from contextlib import ExitStack

import concourse.bass as bass
import concourse.tile as tile
from concourse import bacc, mybir
from concourse.bass_utils import run_bass_kernel_spmd

F32 = mybir.dt.float32
F16 = mybir.dt.float16
ALU = mybir.AluOpType
ACTF = mybir.ActivationFunctionType

B, C, E, F = 8, 2048, 256, 256
EPS = 1e-6

# --- tunables -------------------------------------------------------------
CW = 512            # columns per outer chunk (elementwise granularity)
NCH = C // CW
MMN = 512           # matmul moving free size (== CW)
NBUF = dict(inp=2, bas=2, prod=2, xd=2, tail=1, out=2)


def _mk(ap, dims):
    """Build an AP over the same tensor with explicit [stride, size] free dims."""
    return bass.AP(tensor=ap.tensor, offset=ap.offset, ap=[ap.ap[0]] + dims)


def build_nc():
    nc = bacc.Bacc("TRN2", target_bir_lowering=False, debug=False, num_devices=8)

    for val in (EPS, 1.25 * EPS):
        t = nc.alloc_sbuf_tensor(f"const-f32-{val}", [128, 1], F32)
        nc.gpsimd.memset(t.ap(), val)
        nc.const_aps.aps[(F32, val)] = t.ap()
    nc.all_engine_barrier()

    xp = nc.dram_tensor("xp", [3, E, C], F16, kind="ExternalInput")
    jp = nc.dram_tensor("jp", [3, E, C], F16, kind="ExternalInput")
    wy = nc.dram_tensor("wy", [4, E, F], F16, kind="ExternalInput")  # A^T,(Cw-A)^T,Bw^T,(-Bw)^T
    wt = nc.dram_tensor("wt", [F, F], F16, kind="ExternalInput")     # W^T
    ident = nc.dram_tensor("ident", [128, 128], F16, kind="ExternalInput")
    out = nc.dram_tensor("out", [F, 3, C], F16, kind="ExternalOutput")

    with tile.TileContext(nc) as tc, ExitStack() as ctx:
        wpool = ctx.enter_context(tc.tile_pool(name="w", bufs=1))
        inpool = ctx.enter_context(tc.tile_pool(name="inp", bufs=NBUF["inp"]))
        baspool = ctx.enter_context(tc.tile_pool(name="bas", bufs=NBUF["bas"]))
        prodpool = ctx.enter_context(tc.tile_pool(name="prod", bufs=NBUF["prod"]))
        xdpool = ctx.enter_context(tc.tile_pool(name="xd", bufs=NBUF["xd"]))
        tailpool = ctx.enter_context(tc.tile_pool(name="tail", bufs=NBUF["tail"]))
        outpool = ctx.enter_context(tc.tile_pool(name="outp", bufs=NBUF["out"]))
        ypool = ctx.enter_context(tc.tile_pool(name="ypsum", bufs=2, space="PSUM"))
        dpool = ctx.enter_context(tc.tile_pool(name="dpsum", bufs=2, space="PSUM"))
        rpool = dpool  # dall / dotP / dnsP rotate through the same 2x2-bank slots

        # --- weights: once, resident ---
        wy_sb = []
        for t in range(4):
            w_t = wpool.tile([128, 2, F], F16, tag=f"wy{t}", name=f"wy{t}")
            nc.sync.dma_start(w_t[:], wy[t].rearrange("(k p) f -> p k f", p=128))
            wy_sb.append(w_t)
        wt_sb = wpool.tile([128, 2, F], F16, tag="wt", name="wt")
        nc.sync.dma_start(wt_sb[:], wt.rearrange("(k p) f -> p k f", p=128))
        id_sb = wpool.tile([128, 128], F16, tag="ident", name="ident")
        nc.sync.dma_start(id_sb[:], ident[:, :])

        for ci in range(NCH):
            c0 = ci * CW

            J3 = inpool.tile([128, 3, 2, CW], F16, tag="J3", name="J3")
            X3 = inpool.tile([128, 3, 2, CW], F16, tag="X3", name="X3")
            for i in range(3):
                nc.sync.dma_start(
                    J3[:, i], jp[i][:, c0:c0 + CW].rearrange("(k p) c -> p k c", p=128)
                )
                nc.sync.dma_start(
                    X3[:, i], xp[i][:, c0:c0 + CW].rearrange("(k p) c -> p k c", p=128)
                )
            jz = J3[:, 2]

            def BT(tag, dt=F16, shape=None):
                return baspool.tile(shape or [128, 2, CW], dt, tag=tag, name=tag)

            # --- basis ---
            q123 = BT("q123", shape=[128, 3, 2, CW])
            nc.scalar.activation(q123[:], J3[:], ACTF.Square)
            t1 = BT("t1"); nc.gpsimd.tensor_add(t1[:], q123[:, 0], q123[:, 1])
            n2 = BT("n2"); nc.gpsimd.tensor_add(n2[:], t1[:], q123[:, 2])
            i1 = BT("i1"); nc.scalar.activation(i1[:], n2[:], ACTF.Abs_reciprocal_sqrt)
            jzp = BT("jzp", F32)
            nc.scalar.activation(jzp[:], jz, ACTF.Identity, bias=EPS)
            rD = BT("rD", F32); nc.vector.reciprocal_approx_fast(rD[:], jzp[:])
            g1 = BT("g1", F32); nc.vector.tensor_mul(g1[:], t1[:], rD[:])
            gsq = BT("gsq", F32); nc.scalar.square(gsq[:], g1[:])
            w2 = BT("w2", F32); nc.gpsimd.tensor_add(w2[:], t1[:], gsq[:])
            i3 = BT("i3"); nc.scalar.activation(i3[:], w2[:], ACTF.Abs_reciprocal_sqrt)
            # uznz: [which(uz,nz), k, c]
            uznz = baspool.tile([128, 2, 2, CW], F16, tag="uznz", name="uznz")
            nc.vector.scalar_tensor_tensor(uznz[:, 0], g1[:], -1.0, i3[:], ALU.mult, ALU.mult)
            nc.vector.tensor_mul(uznz[:, 1], jz, i1[:])

            # --- products ---
            # UN6[w, i, kc] = uznz[w] * X3[i]
            UN6 = prodpool.tile([128, 2, 3, 2, CW], F16, tag="UN6", name="UN6")
            KC = 2 * CW
            un_in0 = _mk(uznz[:], [[KC, 2], [0, 3], [1, KC]])
            un_in1 = _mk(X3[:], [[0, 2], [KC, 3], [1, KC]])
            un_out = _mk(UN6[:], [[3 * KC, 2], [KC, 3], [1, KC]])
            nc.vector.tensor_tensor(un_out, un_in0, un_in1, ALU.mult)
            # s = uz*xx + nz*xz
            s = prodpool.tile([128, 2, CW], F16, tag="s", name="s")
            nc.gpsimd.tensor_add(s[:], UN6[:, 0, 0], UN6[:, 1, 2])
            # P02[w] = uznz[w] * s
            P02 = prodpool.tile([128, 2, 2, CW], F16, tag="P02", name="P02")
            p_in0 = _mk(uznz[:], [[KC, 2], [1, KC]])
            p_in1 = _mk(s[:], [[0, 2], [1, KC]])
            p_out = _mk(P02[:], [[KC, 2], [1, KC]])
            nc.vector.tensor_tensor(p_out, p_in0, p_in1, ALU.mult)

            xx, xy, xz = X3[:, 0], X3[:, 1], X3[:, 2]
            P0, P2 = P02[:, 0], P02[:, 1]
            Q0 = UN6[:, 1, 1]; Q2 = UN6[:, 0, 1]
            m3 = UN6[:, 0, 2]; m4 = UN6[:, 1, 0]

            # --- per-comp: Y GEMM -> xall cast -> W GEMM -> dsb cast ---
            terms = {
                0: [(0, xx), (1, P0), (2, Q0)],
                1: [(0, xy), (2, m3), (3, m4)],
                2: [(0, xz), (1, P2), (3, Q2)],
            }
            xall = xdpool.tile([128, 3, 2, CW], F16, tag="xall", name="xall")
            dsb = xdpool.tile([128, 3, 2, CW], F16, tag="dsb", name="dsb")
            for i in range(3):
                yall = ypool.tile([128, 2, MMN], F32, tag="yall", name="yall")
                tl = terms[i]
                n_mm = len(tl) * 2
                for fj in range(2):
                    k = 0
                    for (tw, plane) in tl:
                        for ke in range(2):
                            nc.tensor.matmul(
                                yall[:, fj, :],
                                lhsT=wy_sb[tw][:, ke, fj * 128:(fj + 1) * 128],
                                rhs=plane[:, ke, :],
                                start=(k == 0), stop=(k == n_mm - 1),
                            )
                            k += 1
                nc.vector.tensor_copy(xall[:, i], yall[:])
                dall = dpool.tile([128, 2, MMN], F32, tag="dr", name="dall")
                for fj in range(2):
                    for kg in range(2):
                        nc.tensor.matmul(
                            dall[:, fj, :],
                            lhsT=wt_sb[:, kg, fj * 128:(fj + 1) * 128],
                            rhs=xall[:, i, kg, :],
                            start=(kg == 0), stop=(kg == 1),
                        )
                nc.scalar.copy(dsb[:, i], dall[:])

            # --- VN-LeakyReLU tail ---
            def TT(tag, dt=F16, shape=None):
                return tailpool.tile(shape or [128, 2, CW], dt, tag=tag, name=tag)

            ea = TT("ea", shape=[128, 3, 2, CW])
            nc.vector.tensor_mul(ea[:], dsb[:], dsb[:])
            dva = TT("dva", shape=[128, 3, 2, CW])
            nc.vector.tensor_mul(dva[:], xall[:], dsb[:])
            # dot/dns: 3-way reductions as identity-matmul PSUM accumulation.
            # dns first: dotP reuses the shared psum slot after sqW read dnsP.
            dnsP = rpool.tile([128, 2 * CW], F32, tag="dr", name="dnsP")
            for fj in range(2):
                for i in range(3):
                    nc.tensor.matmul(
                        dnsP[:, fj * CW:(fj + 1) * CW], lhsT=id_sb[:],
                        rhs=ea[:, i, fj, :], start=(i == 0), stop=(i == 2),
                    )
            # inv0 = 0.8/(dns+eps) = 1/sqrt((1.25*dns+1.25*eps)^2)
            sqW = TT("sqW", F32)
            nc.scalar.activation(sqW[:], _mk(dnsP[:], [[CW, 2], [1, CW]]),
                                 ACTF.Square, bias=1.25 * EPS, scale=1.25)
            inv0 = TT("inv0", F32)
            nc.scalar.activation(inv0[:], sqW[:], ACTF.Abs_reciprocal_sqrt)
            dotP = rpool.tile([128, 2 * CW], F32, tag="dr", name="dotP")
            for fj in range(2):
                for i in range(3):
                    nc.tensor.matmul(
                        dotP[:, fj * CW:(fj + 1) * CW], lhsT=id_sb[:],
                        rhs=dva[:, i, fj, :], start=(i == 0), stop=(i == 2),
                    )
            # rr = min(dot,0)*inv0  (<= 0);  out = x - (rr*d)
            rr = TT("rr")
            nc.vector.scalar_tensor_tensor(
                rr[:], _mk(dotP[:], [[CW, 2], [1, CW]]), 0.0, inv0[:], ALU.min, ALU.mult
            )
            ga = TT("ga", shape=[128, 3, 2, CW])
            g_in0 = _mk(rr[:], [[0, 3], [1, KC]])
            g_in1 = _mk(dsb[:], [[KC, 3], [1, KC]])
            g_out = _mk(ga[:], [[KC, 3], [1, KC]])
            nc.vector.tensor_tensor(g_out, g_in0, g_in1, ALU.mult)
            oall = outpool.tile([128, 3, 2, CW], F16, tag="oall", name="oall")
            nc.vector.tensor_sub(oall[:], xall[:], ga[:])
            nc.sync.dma_start(
                out[:, :, c0:c0 + CW].rearrange("(k p) i c -> p i k c", p=128), oall[:]
            )

    nc.compile()
    return nc


_NC_CACHE = {}


def _get_nc():
    if "nc" not in _NC_CACHE:
        _NC_CACHE["nc"] = build_nc()
    return _NC_CACHE["nc"]


def _prep_inputs(X, J, A, Bw, Cw, W):
    A = np.asarray(A, dtype=np.float32)
    Bw = np.asarray(Bw, dtype=np.float32)
    Cw = np.asarray(Cw, dtype=np.float32)
    W = np.asarray(W, dtype=np.float32)
    wy = np.ascontiguousarray(
        np.stack([A.T, (Cw - A).T, Bw.T, (-Bw).T])
    ).astype(np.float16)                       # [4, E, F]
    wt = np.ascontiguousarray(W.T).astype(np.float16)
    ident = np.eye(128, dtype=np.float16)
    in_maps = []
    for b in range(B):
        in_maps.append({
            "xp": np.ascontiguousarray(np.asarray(X[b]).transpose(2, 1, 0)).astype(np.float16),
            "jp": np.ascontiguousarray(np.asarray(J[b]).transpose(2, 1, 0)).astype(np.float16),
            "wy": wy,
            "wt": wt,
            "ident": ident,
        })
    return in_maps


def kernel(X, J, A, Bw, Cw, W):
    in_maps = _prep_inputs(X, J, A, Bw, Cw, W)
    nc = _get_nc()
    try:
        res = run_bass_kernel_spmd(nc, in_maps, core_ids=list(range(B)))
    except Exception:
        import time as _time
        _time.sleep(15)  # transient NRT device errors recover on retry
        res = run_bass_kernel_spmd(nc, in_maps, core_ids=list(range(B)))
    return np.stack([np.asarray(res.results[b]["out"]).astype(np.float32) for b in range(B)])
